# revision 13
# baseline (speedup 1.0000x reference)
"""Trainium2 Bass/Tile kernel for the AttentionModule problem.

Computation (per the reference):
    h_proj  = hidden @ Wa[:, :D].T + ba                       [B, 2E]
    e_proj  = einsum('tbe,fe->tbf', enc, Wa[:, D:])           [T, B, 2E]
    act     = tanh(h_proj + e_proj)
    scores  = einsum('tbf,f->bt', act, w2[0]) (+ b2, dropped — softmax invariant)
    weights = softmax(scores, axis=t)
    applied = einsum('bt,tbe->be', weights, enc)              [B, E]
    out     = tanh(cat(decoder_out, applied) @ Wc.T + bc)     [B, D]

Strategy: data-parallel over batch B=64 across 8 NeuronCores (8 rows each).
All matmul operands are host-pre-transposed so the contraction dim (e/d/k)
lands on SBUF partitions; inputs are cast to bf16 on host (fp32 PSUM accum).

Per-core device layout (f = 2E on PSUM partitions for the big matmul):
    pre[f_tile=128, t=512]  = sum_e WaET[e, f].T @ encT[e, (b,t)]  (8 K-tiles)
    act = tanh(pre + h_projT[f, b])        (ACT, bias = per-partition scalar)
    scores[1, t] += w2[f_tile].T @ act     (PE, M=1, accumulated over 16 f-tiles)
    softmax on one partition, weights broadcast to 128 partitions via DMA
    appliedT[e_tile, b] = reduce_t(encT * wrep)   (DVE scalar_tensor_tensor)
    out[b, :] = tanh(cat(decT, appliedT).T @ WcT + bc)  (PE + ACT)
"""

import numpy as np
import ml_dtypes
from contextlib import ExitStack

import concourse.bass as bass
import concourse.tile as tile
from concourse import bacc, mybir
from concourse.bass_utils import run_bass_kernel_spmd
from concourse.masks import make_identity

B, T, E, D = 64, 512, 1024, 1024
NCORES = 8
BL = B // NCORES          # 8 batch rows per core
F = 2 * E                 # 2048
KE = E // 128             # 8 contraction tiles for e/d
KC = (D + E) // 128       # 16 contraction tiles for the combine matmul
FJ = F // 128             # 16 f-tiles
BF16 = mybir.dt.bfloat16
F32 = mybir.dt.float32
AF = mybir.ActivationFunctionType
ALU = mybir.AluOpType

_nc_cache = None


def _load_consts(tc, ctx, ins, uid=""):
    """Load all weights + encoder states into SBUF. Returns tile dict."""
    nc = tc.nc
    const = ctx.enter_context(tc.tile_pool(name=f"const{uid}", bufs=1))
    tl = {}
    tl["ones"] = const.tile([1, BL], BF16, name="ones", tag="ones")
    nc.vector.memset(tl["ones"], 1.0)
    tl["ident"] = const.tile([128, 128], F32, name="ident", tag="ident")
    make_identity(nc, tl["ident"])

    tl["hT"] = []
    tl["waDT"] = []
    for k in range(KE):
        t_h = const.tile([128, BL], BF16, name=f"hT{k}", tag=f"hT{k}")
        nc.sync.dma_start(out=t_h, in_=ins["hT"][k * 128:(k + 1) * 128, :])
        tl["hT"].append(t_h)
        t_w = const.tile([128, F], BF16, name=f"waDT{k}", tag=f"waDT{k}")
        nc.sync.dma_start(out=t_w, in_=ins["WaDT"][k * 128:(k + 1) * 128, :])
        tl["waDT"].append(t_w)
    tl["ba"] = const.tile([1, F], BF16, name="ba_sb", tag="ba")
    nc.sync.dma_start(out=tl["ba"], in_=ins["baR"])
    tl["waET"] = []
    for k in range(KE):
        t_w = const.tile([128, F], BF16, name=f"waET{k}", tag=f"waET{k}")
        nc.sync.dma_start(out=t_w, in_=ins["WaET"][k * 128:(k + 1) * 128, :])
        tl["waET"].append(t_w)
    tl["w2"] = const.tile([128, FJ], BF16, name="w2_sb", tag="w2")
    nc.sync.dma_start(out=tl["w2"], in_=ins["w2T"])

    # encoder states, transposed: [b, e, t]; per-(k, b) tiles are contiguous
    # 128 KiB blocks, and the first batch column can start computing before
    # the full load lands
    tl["enc"] = [[None] * BL for _ in range(KE)]
    for b in range(BL):
        for k in range(KE):
            t_e = const.tile([128, T], BF16, name=f"enc{k}_{b}", tag=f"enc{k}_{b}")
            nc.sync.dma_start(
                out=t_e,
                in_=ins["encT"][b, k * 128:(k + 1) * 128, :],
            )
            tl["enc"][k][b] = t_e

    tl["decT"] = []
    for k in range(KE):
        t_d = const.tile([128, BL], BF16, name=f"decT{k}", tag=f"decT{k}")
        nc.sync.dma_start(out=t_d, in_=ins["decT"][k * 128:(k + 1) * 128, :])
        tl["decT"].append(t_d)
    tl["wcT"] = []
    for k in range(KC):
        t_w = const.tile([128, D], BF16, name=f"wcT{k}", tag=f"wcT{k}")
        nc.sync.dma_start(out=t_w, in_=ins["WcT"][k * 128:(k + 1) * 128, :])
        tl["wcT"].append(t_w)
    tl["bc"] = const.tile([1, D], BF16, name="bc_sb", tag="bc")
    nc.sync.dma_start(out=tl["bc"], in_=ins["bcR"])
    return tl


def _compute(tc, ctx, tl, wscr, out_d, app_d, uid=""):
    nc = tc.nc
    work = ctx.enter_context(tc.tile_pool(name=f"work{uid}", bufs=1))
    act_pool = ctx.enter_context(tc.tile_pool(name=f"actp{uid}", bufs=3))
    wrep_pool = ctx.enter_context(tc.tile_pool(name=f"wrepp{uid}", bufs=2))
    scr_pool = ctx.enter_context(tc.tile_pool(name=f"scrp{uid}", bufs=2))
    sm_pool = ctx.enter_context(tc.tile_pool(name=f"smp{uid}", bufs=2))
    pe_psum = ctx.enter_context(tc.tile_pool(name=f"pep{uid}", bufs=3, space="PSUM"))
    ps_psum = ctx.enter_context(tc.tile_pool(name=f"psp{uid}", bufs=2, space="PSUM"))
    misc_psum = ctx.enter_context(
        tc.tile_pool(name=f"mip{uid}", bufs=2, space="PSUM"))

    ones, ident = tl["ones"], tl["ident"]

    # ---- h_proj = hidden @ WaD.T + ba  → [BL, F] then transpose ----
    h_proj = work.tile([BL, F], F32, name="h_proj", tag="h_proj")
    for c in range(F // 512):
        ph = misc_psum.tile([BL, 512], F32, name=f"ph{c}", tag="misc")
        for k in range(KE):
            nc.tensor.matmul(
                ph, tl["hT"][k], tl["waDT"][k][:, c * 512:(c + 1) * 512],
                start=(k == 0), stop=False,
            )
        nc.tensor.matmul(
            ph, ones, tl["ba"][:, c * 512:(c + 1) * 512], start=False, stop=True,
        )
        nc.scalar.copy(h_proj[:, c * 512:(c + 1) * 512], ph)

    # h_projT[f, j, b] per-partition bias layout: [128, FJ, BL]
    h_projT = work.tile([128, FJ, BL], F32, name="h_projT", tag="h_projT")
    for j in range(FJ):
        pt = misc_psum.tile([128, BL], F32, name=f"pt{j}", tag="misc")
        nc.tensor.transpose(pt, h_proj[:, j * 128:(j + 1) * 128], ident[0:BL, 0:BL])
        nc.scalar.copy(h_projT[:, j, :], pt)

    # appliedT[e_tile][128, BL] accumulators (fp32)
    appT = []
    for k in range(KE):
        appT.append(work.tile([128, BL], F32, name=f"appT{k}", tag=f"appT{k}"))

    # ---- main loop: per batch row, per f-tile ----
    for b in range(BL):
        ps = ps_psum.tile([1, T], F32, name=f"ps{b}", tag="ps")
        acts = []
        for j in range(FJ):
            pe = pe_psum.tile([128, T], F32, name=f"pe{b}_{j}", tag="pe")
            for k in range(KE):
                nc.tensor.matmul(
                    pe,
                    tl["waET"][k][:, j * 128:(j + 1) * 128],
                    tl["enc"][k][b],
                    start=(k == 0), stop=(k == KE - 1),
                )
            a_t = act_pool.tile([128, T], BF16, name=f"act{b}_{j}", tag="act")
            nc.scalar.activation(a_t, pe, AF.Tanh, bias=h_projT[:, j, b:b + 1])
            acts.append(a_t)
            # emit the score matmul one j behind so PE doesn't stall on tanh
            if j > 0:
                nc.tensor.matmul(ps, tl["w2"][:, j - 1:j], acts[j - 1],
                                 start=(j - 1 == 0), stop=False)
        nc.tensor.matmul(ps, tl["w2"][:, FJ - 1:FJ], acts[FJ - 1],
                         start=False, stop=True)

        # softmax over t on a single partition
        negmax = sm_pool.tile([1, 1], F32, name=f"negmax{b}", tag="negmax")
        nc.vector.reduce_max(negmax, ps, axis=mybir.AxisListType.X, negate=True)
        wexp = sm_pool.tile([1, T], F32, name=f"wexp{b}", tag="wexp")
        sume = sm_pool.tile([1, 1], F32, name=f"sume{b}", tag="sume")
        nc.scalar.activation(wexp, ps, AF.Exp, bias=negmax, accum_out=sume)
        rsum = sm_pool.tile([1, 1], F32, name=f"rsum{b}", tag="rsum")
        nc.vector.reciprocal(rsum, sume)
        wnorm = sm_pool.tile([1, T], BF16, name=f"wnorm{b}", tag="wnorm")
        nc.vector.tensor_scalar_mul(wnorm, wexp, rsum)

        # broadcast weights to 128 partitions via DRAM round-trip
        nc.sync.dma_start(out=wscr[b:b + 1, :], in_=wnorm)
        wrep = wrep_pool.tile([128, T], BF16, name=f"wrep{b}", tag="wrep")
        row = wscr[b:b + 1, :]
        bsrc = bass.AP(tensor=row.tensor, offset=row.offset,
                       ap=[[0, 128]] + [list(p) for p in row.ap[1:]])
        nc.sync.dma_start(out=wrep, in_=bsrc)

        # appliedT[:, b] = sum_t enc * w
        for k in range(KE):
            scr = scr_pool.tile([128, T], BF16, name=f"scr{b}_{k}", tag="scr")
            nc.vector.scalar_tensor_tensor(
                out=scr, in0=tl["enc"][k][b], scalar=1.0, in1=wrep,
                op0=ALU.mult, op1=ALU.mult,
                accum_out=appT[k][:, b:b + 1],
            )

    # ---- epilogue: applied output + combine matmul ----
    applied_sb = work.tile([BL, E], F32, name="applied_sb", tag="applied_sb")
    appT_bf = []
    for k in range(KE):
        t_c = work.tile([128, BL], BF16, name=f"appBf{k}", tag=f"appBf{k}")
        nc.vector.tensor_copy(t_c, appT[k])
        appT_bf.append(t_c)
        pa = misc_psum.tile([BL, 128], F32, name=f"pa{k}", tag="misc")
        nc.tensor.transpose(pa, appT[k], ident)
        nc.scalar.copy(applied_sb[:, k * 128:(k + 1) * 128], pa)
    nc.sync.dma_start(out=app_d, in_=applied_sb)

    out_sb = work.tile([BL, D], F32, name="out_sb", tag="out_sb")
    for h in range(D // 512):
        pc = misc_psum.tile([BL, 512], F32, name=f"pc{h}", tag="misc")
        for k in range(KC):
            lhs = tl["decT"][k] if k < KE else appT_bf[k - KE]
            nc.tensor.matmul(
                pc, lhs, tl["wcT"][k][:, h * 512:(h + 1) * 512],
                start=(k == 0), stop=False,
            )
        nc.tensor.matmul(
            pc, ones, tl["bc"][:, h * 512:(h + 1) * 512], start=False, stop=True,
        )
        nc.scalar.activation(out_sb[:, h * 512:(h + 1) * 512], pc, AF.Tanh)
    nc.sync.dma_start(out=out_d, in_=out_sb)


def build_nc(reps=1, mode="full"):
    """mode: 'full' = load+compute per rep; 'compute1' = load once, compute
    `reps` times; 'dma' = load only, `reps` times."""
    nc = bacc.Bacc("TRN2", target_bir_lowering=False, debug=False)
    ins = {}

    def din(name, shape, dt=BF16):
        ins[name] = nc.dram_tensor(name, shape, dt, kind="ExternalInput").ap()

    din("encT", [BL, E, T])
    din("hT", [D, BL])
    din("decT", [D, BL])
    din("WaDT", [D, F])
    din("WaET", [E, F])
    din("WcT", [D + E, D])
    din("w2T", [128, FJ])
    din("baR", [1, F])
    din("bcR", [1, D])
    wscr = nc.dram_tensor("wscr", [BL, T], BF16, kind="Internal").ap()
    out_d = nc.dram_tensor("out", [BL, D], F32, kind="ExternalOutput").ap()
    app_d = nc.dram_tensor("applied", [BL, E], F32, kind="ExternalOutput").ap()
    with tile.TileContext(nc) as tc:
        if mode == "full":
            for r in range(reps):
                with ExitStack() as ctx:
                    tl = _load_consts(tc, ctx, ins, uid=f"r{r}")
                    _compute(tc, ctx, tl, wscr, out_d, app_d, uid=f"r{r}")
        elif mode == "compute1":
            with ExitStack() as octx:
                tl = _load_consts(tc, octx, ins)
                for r in range(reps):
                    with ExitStack() as ctx:
                        _compute(tc, ctx, tl, wscr, out_d, app_d, uid=f"r{r}")
        elif mode == "dma":
            for r in range(reps):
                with ExitStack() as ctx:
                    tl = _load_consts(tc, ctx, ins, uid=f"r{r}")
                    # touch one tile so loads aren't dead-code eliminated
                    s = ctx.enter_context(tc.tile_pool(name=f"s{r}", bufs=1))
                    acc = s.tile([128, 1], F32, name="acc", tag="acc")
                    touch = ([t for row in tl["enc"] for t in row]
                             + tl["waDT"] + tl["waET"] + tl["wcT"] + tl["hT"]
                             + tl["decT"])
                    for i, t in enumerate(touch):
                        nc.vector.reduce_max(acc, t[:, 0:1],
                                             axis=mybir.AxisListType.X)
                    nc.sync.dma_start(out=out_d[0:1, 0:128],
                                      in_=acc.rearrange("p one -> one p"))
        else:
            raise ValueError(mode)
    nc.compile()
    return nc


def _prep_inputs(hidden, decoder_out, encoder_states, Wa, ba, w2, Wc, bc):
    bf = ml_dtypes.bfloat16
    f32 = np.float32

    def to_bf(a):
        return np.ascontiguousarray(np.asarray(a, f32)).astype(bf)

    shared = {
        "WaDT": np.ascontiguousarray(np.asarray(Wa[:, :D], f32).T).astype(bf),
        "WaET": np.ascontiguousarray(np.asarray(Wa[:, D:], f32).T).astype(bf),
        "WcT": np.ascontiguousarray(np.asarray(Wc, f32).T).astype(bf),
        "w2T": np.ascontiguousarray(
            np.asarray(w2[0], f32).reshape(FJ, 128).T).astype(bf),
        "baR": to_bf(np.asarray(ba, f32).reshape(1, F)),
        "bcR": to_bf(np.asarray(bc, f32).reshape(1, D)),
    }
    enc_bf = np.asarray(encoder_states, f32).astype(bf)  # [T, B, E]
    in_maps = []
    for c in range(NCORES):
        sl = slice(c * BL, (c + 1) * BL)
        encT = np.ascontiguousarray(enc_bf[:, sl, :].transpose(1, 2, 0))
        m = dict(shared)
        m["encT"] = encT
        m["hT"] = np.ascontiguousarray(np.asarray(hidden[sl], f32).T).astype(bf)
        m["decT"] = np.ascontiguousarray(np.asarray(decoder_out[sl], f32).T).astype(bf)
        in_maps.append(m)
    return in_maps


def kernel(hidden, decoder_out, encoder_states, Wa, ba, w2, b2, Wc, bc):
    global _nc_cache
    if _nc_cache is None:
        _nc_cache = build_nc()
    in_maps = _prep_inputs(hidden, decoder_out, encoder_states, Wa, ba, w2, Wc, bc)
    res = run_bass_kernel_spmd(_nc_cache, in_maps, core_ids=list(range(NCORES)))
    out = np.concatenate([res.results[c]["out"] for c in range(NCORES)], axis=0)
    applied = np.concatenate(
        [res.results[c]["applied"] for c in range(NCORES)], axis=0)
    return out.astype(np.float32), applied.astype(np.float32)


# revision 33
# speedup vs baseline: 1.3902x; 1.3902x over previous
"""Trainium2 Bass/Tile kernel for the AttentionModule problem.

Computation (per the reference):
    h_proj  = hidden @ Wa[:, :D].T + ba                       [B, 2E]
    e_proj  = einsum('tbe,fe->tbf', enc, Wa[:, D:])           [T, B, 2E]
    act     = tanh(h_proj + e_proj)
    scores  = einsum('tbf,f->bt', act, w2[0]) (+ b2, dropped — softmax invariant)
    weights = softmax(scores, axis=t)
    applied = einsum('bt,tbe->be', weights, enc)              [B, E]
    out     = tanh(cat(decoder_out, applied) @ Wc.T + bc)     [B, D]

Strategy: data-parallel over batch B=64 across 8 NeuronCores (8 rows each).
All matmul operands are host-pre-transposed so the contraction dim (e/d/k)
lands on SBUF partitions; inputs are cast to bf16 on host (fp32 PSUM accum).

Per-core device layout (f = 2E on PSUM partitions for the big matmul):
    pre[f_tile=128, t=512]  = sum_e WaET[e, f].T @ encT[e, (b,t)]  (8 K-tiles)
    act = tanh(pre + h_projT[f, b])        (ACT, bias = per-partition scalar)
    scores[1, t] += w2[f_tile].T @ act     (PE, M=1, accumulated over 16 f-tiles)
    softmax on one partition, weights broadcast to 128 partitions via DMA
    appliedT[e_tile, b] = reduce_t(encT * wrep)   (DVE scalar_tensor_tensor)
    out[b, :] = tanh(cat(decT, appliedT).T @ WcT + bc)  (PE + ACT)
"""

import numpy as np
import ml_dtypes
from contextlib import ExitStack

import concourse.bass as bass
import concourse.tile as tile
from concourse import bacc, mybir
from concourse.bass_utils import run_bass_kernel_spmd
from concourse.masks import make_identity

B, T, E, D = 64, 512, 1024, 1024
NCORES = 8
BL = B // NCORES          # 8 batch rows per core
F = 2 * E                 # 2048
KE = E // 128             # 8 contraction tiles for e/d
KC = (D + E) // 128       # 16 contraction tiles for the combine matmul
FJ = F // 128             # 16 f-tiles
BF16 = mybir.dt.bfloat16
F32 = mybir.dt.float32
AF = mybir.ActivationFunctionType
ALU = mybir.AluOpType

_nc_cache = None


def _load_consts(tc, ctx, ins, uid=""):
    """Load all weights + encoder states into SBUF. Returns tile dict."""
    nc = tc.nc
    const = ctx.enter_context(tc.tile_pool(name=f"const{uid}", bufs=1))
    tl = {}
    tl["ones"] = const.tile([1, BL], BF16, name="ones", tag="ones")
    nc.vector.memset(tl["ones"], 1.0)
    tl["ident"] = const.tile([128, 128], F32, name="ident", tag="ident")
    make_identity(nc, tl["ident"])

    tl["hT"] = []
    tl["waDT"] = []
    for k in range(KE):
        t_h = const.tile([128, BL], BF16, name=f"hT{k}", tag=f"hT{k}")
        nc.sync.dma_start(out=t_h, in_=ins["hT"][k * 128:(k + 1) * 128, :])
        tl["hT"].append(t_h)
        t_w = const.tile([128, F], BF16, name=f"waDT{k}", tag=f"waDT{k}")
        nc.sync.dma_start(out=t_w, in_=ins["WaDT"][k * 128:(k + 1) * 128, :])
        tl["waDT"].append(t_w)
    tl["ba"] = const.tile([1, F], BF16, name="ba_sb", tag="ba")
    nc.sync.dma_start(out=tl["ba"], in_=ins["baR"])
    tl["w2"] = const.tile([128, FJ], BF16, name="w2_sb", tag="w2")
    nc.sync.dma_start(out=tl["w2"], in_=ins["w2T"])
    tl["waET"] = []
    for k in range(KE):
        t_w = const.tile([128, F], BF16, name=f"waET{k}", tag=f"waET{k}")
        nc.sync.dma_start(out=t_w, in_=ins["WaET"][k * 128:(k + 1) * 128, :])
        tl["waET"].append(t_w)

    # encoder states, transposed: [b, e, t]; per-(k, b) tiles are contiguous
    # 128 KiB blocks, and the first batch column can start computing before
    # the full load lands
    tl["enc"] = [[None] * BL for _ in range(KE)]
    for b in range(BL):
        for k in range(KE):
            t_e = const.tile([128, T], BF16, name=f"enc{k}_{b}", tag=f"enc{k}_{b}")
            nc.sync.dma_start(
                out=t_e,
                in_=ins["encT"][b, k * 128:(k + 1) * 128, :],
            )
            tl["enc"][k][b] = t_e

    tl["decT"] = []
    for k in range(KE):
        t_d = const.tile([128, BL], BF16, name=f"decT{k}", tag=f"decT{k}")
        nc.sync.dma_start(out=t_d, in_=ins["decT"][k * 128:(k + 1) * 128, :])
        tl["decT"].append(t_d)
    tl["wcT"] = []
    for k in range(KC):
        t_w = const.tile([128, D], BF16, name=f"wcT{k}", tag=f"wcT{k}")
        nc.sync.dma_start(out=t_w, in_=ins["WcT"][k * 128:(k + 1) * 128, :])
        tl["wcT"].append(t_w)
    tl["bc"] = const.tile([1, D], BF16, name="bc_sb", tag="bc")
    nc.sync.dma_start(out=tl["bc"], in_=ins["bcR"])
    return tl


def _compute(tc, ctx, tl, wscr, out_d, app_d, uid=""):
    nc = tc.nc
    work = ctx.enter_context(tc.tile_pool(name=f"work{uid}", bufs=1))
    act_pool = ctx.enter_context(tc.tile_pool(name=f"actp{uid}", bufs=3))
    wrep_pool = ctx.enter_context(tc.tile_pool(name=f"wrepp{uid}", bufs=2))
    scr_pool = ctx.enter_context(tc.tile_pool(name=f"scrp{uid}", bufs=2))
    sm_pool = ctx.enter_context(tc.tile_pool(name=f"smp{uid}", bufs=2))
    pe_psum = ctx.enter_context(tc.tile_pool(name=f"pep{uid}", bufs=3, space="PSUM"))
    ps_psum = ctx.enter_context(tc.tile_pool(name=f"psp{uid}", bufs=2, space="PSUM"))
    misc_psum = ctx.enter_context(
        tc.tile_pool(name=f"mip{uid}", bufs=1, space="PSUM"))
    pc_psum = ctx.enter_context(
        tc.tile_pool(name=f"pcp{uid}", bufs=2, space="PSUM"))

    ones, ident = tl["ones"], tl["ident"]

    # ---- h_proj = hidden @ WaD.T + ba  → [BL, F] then transpose ----
    h_proj = work.tile([BL, F], F32, name="h_proj", tag="h_proj")
    for c in range(F // 512):
        ph = misc_psum.tile([BL, 512], F32, name=f"ph{c}", tag="misc")
        for k in range(KE):
            nc.tensor.matmul(
                ph, tl["hT"][k], tl["waDT"][k][:, c * 512:(c + 1) * 512],
                start=(k == 0), stop=False,
            )
        nc.tensor.matmul(
            ph, ones, tl["ba"][:, c * 512:(c + 1) * 512], start=False, stop=True,
        )
        nc.scalar.copy(h_proj[:, c * 512:(c + 1) * 512], ph)

    # h_projT[f, j, b] per-partition bias layout: [128, FJ, BL]
    h_projT = work.tile([128, FJ, BL], F32, name="h_projT", tag="h_projT")
    for j in range(FJ):
        pt = misc_psum.tile([128, BL], F32, name=f"pt{j}", tag="misc")
        nc.tensor.transpose(pt, h_proj[:, j * 128:(j + 1) * 128], ident[0:BL, 0:BL])
        nc.scalar.copy(h_projT[:, j, :], pt)

    # appliedT[e_tile][128, BL] accumulators (fp32)
    appT = []
    for k in range(KE):
        appT.append(work.tile([128, BL], F32, name=f"appT{k}", tag=f"appT{k}"))

    pcs = []

    # ---- main loop: per batch row, per f-tile ----
    for b in range(BL):
        ps = ps_psum.tile([1, T], F32, name=f"ps{b}", tag="ps")
        acts = []
        for j in range(FJ):
            pe = pe_psum.tile([128, T], F32, name=f"pe{b}_{j}", tag="pe")
            for k in range(KE):
                nc.tensor.matmul(
                    pe,
                    tl["waET"][k][:, j * 128:(j + 1) * 128],
                    tl["enc"][k][b],
                    start=(k == 0), stop=(k == KE - 1),
                )
            a_t = act_pool.tile([128, T], BF16, name=f"act{b}_{j}", tag="act")
            nc.scalar.activation(a_t, pe, AF.Tanh, bias=h_projT[:, j, b:b + 1])
            acts.append(a_t)
            # emit the score matmul one j behind so PE doesn't stall on tanh
            if j > 0:
                nc.tensor.matmul(ps, tl["w2"][:, j - 1:j], acts[j - 1],
                                 start=(j - 1 == 0), stop=False)
        nc.tensor.matmul(ps, tl["w2"][:, FJ - 1:FJ], acts[FJ - 1],
                         start=False, stop=True)

        # softmax over t on a single partition
        negmax = sm_pool.tile([1, 1], F32, name=f"negmax{b}", tag="negmax")
        nc.vector.reduce_max(negmax, ps, axis=mybir.AxisListType.X, negate=True)
        wexp = sm_pool.tile([1, T], F32, name=f"wexp{b}", tag="wexp")
        sume = sm_pool.tile([1, 1], F32, name=f"sume{b}", tag="sume")
        nc.scalar.activation(wexp, ps, AF.Exp, bias=negmax, accum_out=sume)
        rsum = sm_pool.tile([1, 1], F32, name=f"rsum{b}", tag="rsum")
        nc.vector.reciprocal(rsum, sume)
        wnorm = sm_pool.tile([1, T], BF16, name=f"wnorm{b}", tag="wnorm")
        nc.vector.tensor_scalar_mul(wnorm, wexp, rsum)

        # broadcast weights to 128 partitions via DRAM round-trip
        nc.sync.dma_start(out=wscr[b:b + 1, :], in_=wnorm)
        wrep = wrep_pool.tile([128, T], BF16, name=f"wrep{b}", tag="wrep")
        row = wscr[b:b + 1, :]
        bsrc = bass.AP(tensor=row.tensor, offset=row.offset,
                       ap=[[0, 128]] + [list(p) for p in row.ap[1:]])
        nc.sync.dma_start(out=wrep, in_=bsrc)

        # combine matmul, decoder half: depends only on decT/WcT; emit it
        # mid-loop (weights loaded by now, PE in-order) to keep it off the
        # tail — the applied half and bias finish in the epilogue
        if b == 2:
            for h in range(D // 512):
                pc = pc_psum.tile([BL, 512], F32, name=f"pc{h}", tag="pc")
                for k in range(KE):
                    nc.tensor.matmul(
                        pc, tl["decT"][k],
                        tl["wcT"][k][:, h * 512:(h + 1) * 512],
                        start=(k == 0), stop=False,
                    )
                pcs.append(pc)
        # appliedT[:, b] = sum_t enc * w
        for k in range(KE):
            scr = scr_pool.tile([128, T], BF16, name=f"scr{b}_{k}", tag="scr")
            nc.vector.scalar_tensor_tensor(
                out=scr, in0=tl["enc"][k][b], scalar=1.0, in1=wrep,
                op0=ALU.mult, op1=ALU.mult,
                accum_out=appT[k][:, b:b + 1],
            )

    # ---- epilogue: applied output + combine matmul ----
    applied_sb = work.tile([BL, E], F32, name="applied_sb", tag="applied_sb")
    appT_bf = []
    for k in range(KE):
        t_c = work.tile([128, BL], BF16, name=f"appBf{k}", tag=f"appBf{k}")
        nc.vector.tensor_copy(t_c, appT[k])
        appT_bf.append(t_c)
        pa = misc_psum.tile([BL, 128], F32, name=f"pa{k}", tag="misc")
        nc.tensor.transpose(pa, appT[k], ident)
        nc.scalar.copy(applied_sb[:, k * 128:(k + 1) * 128], pa)
    nc.sync.dma_start(out=app_d, in_=applied_sb)

    out_sb = work.tile([BL, D], F32, name="out_sb", tag="out_sb")
    for h in range(D // 512):
        pc = pcs[h]
        for k in range(KE, KC):
            nc.tensor.matmul(
                pc, appT_bf[k - KE], tl["wcT"][k][:, h * 512:(h + 1) * 512],
                start=False, stop=False,
            )
        nc.tensor.matmul(
            pc, ones, tl["bc"][:, h * 512:(h + 1) * 512], start=False, stop=True,
        )
        nc.scalar.activation(out_sb[:, h * 512:(h + 1) * 512], pc, AF.Tanh)
    nc.sync.dma_start(out=out_d, in_=out_sb)


def _compute_b(tc, ctx, tl, ins, wscr, hscr, out_d, app_d, uid=""):
    """Layout-B main loop: t on PSUM partitions; scores via DVE free-dim
    reduce instead of 128 M=1 PE matmuls (saves ~27us of PE time)."""
    nc = tc.nc
    TT = T // 128             # 4 t-tiles per batch row
    FC = F // 512             # 4 f-chunks
    work = ctx.enter_context(tc.tile_pool(name=f"work{uid}", bufs=1))
    act_pool = ctx.enter_context(tc.tile_pool(name=f"actp{uid}", bufs=3))
    hrep_pool = ctx.enter_context(tc.tile_pool(name=f"hrepp{uid}", bufs=1))
    wrep_pool = ctx.enter_context(tc.tile_pool(name=f"wrepp{uid}", bufs=2))
    scr_pool = ctx.enter_context(tc.tile_pool(name=f"scrp{uid}", bufs=2))
    sm_pool = ctx.enter_context(tc.tile_pool(name=f"smp{uid}", bufs=2))
    pe_psum = ctx.enter_context(tc.tile_pool(name=f"pep{uid}", bufs=4, space="PSUM"))
    misc_psum = ctx.enter_context(
        tc.tile_pool(name=f"mip{uid}", bufs=2, space="PSUM"))

    ones, ident = tl["ones"], tl["ident"]

    # w2 broadcast to all partitions once: [128, F] bf16
    w2rep = work.tile([128, F], BF16, name="w2rep", tag="w2rep")
    w2r = ins["w2R"]
    nc.sync.dma_start(out=w2rep, in_=bass.AP(
        tensor=w2r.tensor, offset=w2r.offset,
        ap=[[0, 128]] + [list(p) for p in w2r.ap[1:]]))

    # ---- h_proj = hidden @ WaD.T + ba → [BL, F] bf16, parked in DRAM ----
    h_proj = work.tile([BL, F], BF16, name="h_proj", tag="h_proj")
    for c in range(FC):
        ph = misc_psum.tile([BL, 512], F32, name=f"ph{c}", tag="misc")
        for k in range(KE):
            nc.tensor.matmul(
                ph, tl["hT"][k], tl["waDT"][k][:, c * 512:(c + 1) * 512],
                start=(k == 0), stop=False,
            )
        nc.tensor.matmul(
            ph, ones, tl["ba"][:, c * 512:(c + 1) * 512], start=False, stop=True,
        )
        nc.scalar.copy(h_proj[:, c * 512:(c + 1) * 512], ph)
    nc.sync.dma_start(out=hscr, in_=h_proj)

    appT = []
    for k in range(KE):
        appT.append(work.tile([128, BL], F32, name=f"appT{k}", tag=f"appT{k}"))
    scores_sb = work.tile([128, BL * TT], F32, name="scores_sb", tag="scores_sb")

    # ---- main loop ----
    for b in range(BL):
        # h_rep[fc]: h_proj row b broadcast to 128 partitions (reused by tt)
        hreps = []
        for fc in range(FC):
            hr = hrep_pool.tile([128, 512], BF16, name=f"hrep{b}_{fc}",
                                tag=f"hrep{fc}")
            src = hscr[b:b + 1, fc * 512:(fc + 1) * 512]
            nc.sync.dma_start(out=hr, in_=bass.AP(
                tensor=src.tensor, offset=src.offset,
                ap=[[0, 128]] + [list(p) for p in src.ap[1:]]))
            hreps.append(hr)
        for tt in range(TT):
            tsl = slice(tt * 128, (tt + 1) * 128)
            sc_parts = scr_pool.tile([128, FC], F32, name=f"scp{b}_{tt}",
                                     tag="sc_parts")
            for fc in range(FC):
                pre = pe_psum.tile([128, 512], F32, name=f"pre{b}_{tt}_{fc}",
                                   tag="pe")
                for k in range(KE):
                    nc.tensor.matmul(
                        pre,
                        tl["enc"][k][b][:, tsl],
                        tl["waET"][k][:, fc * 512:(fc + 1) * 512],
                        start=(k == 0), stop=(k == KE - 1),
                    )
                nc.vector.tensor_add(pre, pre, hreps[fc])
                a_t = act_pool.tile([128, 512], BF16, name=f"act{b}_{tt}_{fc}",
                                    tag="act")
                nc.scalar.activation(a_t, pre, AF.Tanh)
                scr = scr_pool.tile([128, 512], BF16, name=f"scr{b}_{tt}_{fc}",
                                    tag="scr")
                nc.vector.scalar_tensor_tensor(
                    out=scr, in0=a_t, scalar=1.0,
                    in1=w2rep[:, fc * 512:(fc + 1) * 512],
                    op0=ALU.mult, op1=ALU.mult,
                    accum_out=sc_parts[:, fc:fc + 1],
                )
            nc.vector.reduce_sum(scores_sb[:, b * TT + tt:b * TT + tt + 1],
                                 sc_parts, axis=mybir.AxisListType.X)

        # transpose scores [128, TT] -> [1, T] on one partition
        srow = sm_pool.tile([1, T], F32, name=f"srow{b}", tag="srow")
        for tt in range(TT):
            pr = misc_psum.tile([1, 128], F32, name=f"pr{b}_{tt}", tag="misc")
            nc.tensor.transpose(
                pr, scores_sb[:, b * TT + tt:b * TT + tt + 1], ident)
            nc.scalar.copy(srow[:, tt * 128:(tt + 1) * 128], pr)

        # softmax over t on a single partition
        negmax = sm_pool.tile([1, 1], F32, name=f"negmax{b}", tag="negmax")
        nc.vector.reduce_max(negmax, srow, axis=mybir.AxisListType.X, negate=True)
        wexp = sm_pool.tile([1, T], F32, name=f"wexp{b}", tag="wexp")
        sume = sm_pool.tile([1, 1], F32, name=f"sume{b}", tag="sume")
        nc.scalar.activation(wexp, srow, AF.Exp, bias=negmax, accum_out=sume)
        rsum = sm_pool.tile([1, 1], F32, name=f"rsum{b}", tag="rsum")
        nc.vector.reciprocal(rsum, sume)
        wnorm = sm_pool.tile([1, T], BF16, name=f"wnorm{b}", tag="wnorm")
        nc.vector.tensor_scalar_mul(wnorm, wexp, rsum)

        nc.sync.dma_start(out=wscr[b:b + 1, :], in_=wnorm)
        wrep = wrep_pool.tile([128, T], BF16, name=f"wrep{b}", tag="wrep")
        row = wscr[b:b + 1, :]
        nc.sync.dma_start(out=wrep, in_=bass.AP(
            tensor=row.tensor, offset=row.offset,
            ap=[[0, 128]] + [list(p) for p in row.ap[1:]]))

        for k in range(KE):
            sca = scr_pool.tile([128, T], BF16, name=f"sca{b}_{k}", tag="scr")
            nc.vector.scalar_tensor_tensor(
                out=sca, in0=tl["enc"][k][b], scalar=1.0, in1=wrep,
                op0=ALU.mult, op1=ALU.mult,
                accum_out=appT[k][:, b:b + 1],
            )

    # ---- epilogue: applied output + combine matmul ----
    applied_sb = work.tile([BL, E], F32, name="applied_sb", tag="applied_sb")
    appT_bf = []
    for k in range(KE):
        t_c = work.tile([128, BL], BF16, name=f"appBf{k}", tag=f"appBf{k}")
        nc.vector.tensor_copy(t_c, appT[k])
        appT_bf.append(t_c)
        pa = misc_psum.tile([BL, 128], F32, name=f"pa{k}", tag="misc")
        nc.tensor.transpose(pa, appT[k], ident)
        nc.scalar.copy(applied_sb[:, k * 128:(k + 1) * 128], pa)
    nc.sync.dma_start(out=app_d, in_=applied_sb)

    out_sb = work.tile([BL, D], F32, name="out_sb", tag="out_sb")
    for h in range(D // 512):
        pc = misc_psum.tile([BL, 512], F32, name=f"pc{h}", tag="misc")
        for k in range(KC):
            lhs = tl["decT"][k] if k < KE else appT_bf[k - KE]
            nc.tensor.matmul(
                pc, lhs, tl["wcT"][k][:, h * 512:(h + 1) * 512],
                start=(k == 0), stop=False,
            )
        nc.tensor.matmul(
            pc, ones, tl["bc"][:, h * 512:(h + 1) * 512], start=False, stop=True,
        )
        nc.scalar.activation(out_sb[:, h * 512:(h + 1) * 512], pc, AF.Tanh)
    nc.sync.dma_start(out=out_d, in_=out_sb)


def build_nc(reps=1, mode="full"):
    """mode: 'full' = load+compute per rep; 'compute1' = load once, compute
    `reps` times; 'dma' = load only, `reps` times."""
    nc = bacc.Bacc("TRN2", target_bir_lowering=False, debug=False)
    ins = {}

    def din(name, shape, dt=BF16):
        ins[name] = nc.dram_tensor(name, shape, dt, kind="ExternalInput").ap()

    din("encT", [BL, E, T])
    din("hT", [D, BL])
    din("decT", [D, BL])
    din("WaDT", [D, F])
    din("WaET", [E, F])
    din("WcT", [D + E, D])
    din("w2T", [128, FJ])
    din("w2R", [1, F])
    din("baR", [1, F])
    din("bcR", [1, D])
    wscr = nc.dram_tensor("wscr", [BL, T], BF16, kind="Internal").ap()
    hscr = nc.dram_tensor("hscr", [BL, F], BF16, kind="Internal").ap()
    out_d = nc.dram_tensor("out", [BL, D], F32, kind="ExternalOutput").ap()
    app_d = nc.dram_tensor("applied", [BL, E], F32, kind="ExternalOutput").ap()
    with tile.TileContext(nc) as tc:
        if mode == "full_b":
            for r in range(reps):
                with ExitStack() as ctx:
                    tl = _load_consts(tc, ctx, ins, uid=f"r{r}")
                    _compute_b(tc, ctx, tl, ins, wscr, hscr, out_d, app_d,
                               uid=f"r{r}")
        elif mode == "full":
            for r in range(reps):
                with ExitStack() as ctx:
                    tl = _load_consts(tc, ctx, ins, uid=f"r{r}")
                    _compute(tc, ctx, tl, wscr, out_d, app_d, uid=f"r{r}")
        elif mode == "compute1":
            with ExitStack() as octx:
                tl = _load_consts(tc, octx, ins)
                for r in range(reps):
                    with ExitStack() as ctx:
                        _compute(tc, ctx, tl, wscr, out_d, app_d, uid=f"r{r}")
        elif mode == "dma":
            for r in range(reps):
                with ExitStack() as ctx:
                    tl = _load_consts(tc, ctx, ins, uid=f"r{r}")
                    # touch one tile so loads aren't dead-code eliminated
                    s = ctx.enter_context(tc.tile_pool(name=f"s{r}", bufs=1))
                    acc = s.tile([128, 1], F32, name="acc", tag="acc")
                    touch = ([t for row in tl["enc"] for t in row]
                             + tl["waDT"] + tl["waET"] + tl["wcT"] + tl["hT"]
                             + tl["decT"])
                    for i, t in enumerate(touch):
                        nc.vector.reduce_max(acc, t[:, 0:1],
                                             axis=mybir.AxisListType.X)
                    nc.sync.dma_start(out=out_d[0:1, 0:128],
                                      in_=acc.rearrange("p one -> one p"))
        else:
            raise ValueError(mode)
    nc.compile()
    return nc


def _prep_inputs(hidden, decoder_out, encoder_states, Wa, ba, w2, Wc, bc):
    bf = ml_dtypes.bfloat16
    f32 = np.float32

    def to_bf(a):
        return np.ascontiguousarray(np.asarray(a, f32)).astype(bf)

    shared = {
        "WaDT": np.ascontiguousarray(np.asarray(Wa[:, :D], f32).T).astype(bf),
        "WaET": np.ascontiguousarray(np.asarray(Wa[:, D:], f32).T).astype(bf),
        "WcT": np.ascontiguousarray(np.asarray(Wc, f32).T).astype(bf),
        "w2T": np.ascontiguousarray(
            np.asarray(w2[0], f32).reshape(FJ, 128).T).astype(bf),
        "w2R": to_bf(np.asarray(w2[0], f32).reshape(1, F)),
        "baR": to_bf(np.asarray(ba, f32).reshape(1, F)),
        "bcR": to_bf(np.asarray(bc, f32).reshape(1, D)),
    }
    enc_bf = np.asarray(encoder_states, f32).astype(bf)  # [T, B, E]
    in_maps = []
    for c in range(NCORES):
        sl = slice(c * BL, (c + 1) * BL)
        encT = np.ascontiguousarray(enc_bf[:, sl, :].transpose(1, 2, 0))
        m = dict(shared)
        m["encT"] = encT
        m["hT"] = np.ascontiguousarray(np.asarray(hidden[sl], f32).T).astype(bf)
        m["decT"] = np.ascontiguousarray(np.asarray(decoder_out[sl], f32).T).astype(bf)
        in_maps.append(m)
    return in_maps


def kernel(hidden, decoder_out, encoder_states, Wa, ba, w2, b2, Wc, bc):
    global _nc_cache
    if _nc_cache is None:
        _nc_cache = build_nc()
    in_maps = _prep_inputs(hidden, decoder_out, encoder_states, Wa, ba, w2, Wc, bc)
    res = run_bass_kernel_spmd(_nc_cache, in_maps, core_ids=list(range(NCORES)))
    out = np.concatenate([res.results[c]["out"] for c in range(NCORES)], axis=0)
    applied = np.concatenate(
        [res.results[c]["applied"] for c in range(NCORES)], axis=0)
    return out.astype(np.float32), applied.astype(np.float32)
